# revision 13
# baseline (speedup 1.0000x reference)
"""BiLSTM-CRF Trainium2 kernel (f32 end-to-end for exact Viterbi tags).

Self-contained: hardcodes V=100000, H=256, T=11, B=32, S=512 and an 8-way
batch shard (4 rows/core). Each core runs the full pipeline for its 4 batch
rows with zero cross-core communication:

  P1 embedding gather (indirect DMA, f32 table) -> PE transposes -> xeT
  P2 xg0 = wih0 @ xe.T + b   (bulk f32 PE matmuls) -> DRAM
  P3 layer-0 fwd+bwd LSTM scans (interleaved; xg prefetched 16 steps/DMA)
  P4 xg1 = wih1 @ [h0f;h0b].T + b -> DRAM
  P5 layer-1 fwd+bwd scans
  P6 feats = w_out @ h1cat.T + b_out -> PE transpose -> per-batch layout
  P7 Viterbi max-plus forward + backward scans (DVE)
  P8 score = max terminal; tags via max-marginal argmax (bulk DVE)

Gate order is host-permuted from PyTorch (i,f,g,o) to (i,f,o,g) so one
sigmoid covers i,f,o contiguously and tanh(g) reads PSUM directly.
"""

import os
import sys
from contextlib import ExitStack

for _p in ("/opt/trn_rl_repo", "/root/.axon_site/_ro/trn_rl_repo"):
    if os.path.isdir(_p) and _p not in sys.path:
        sys.path.insert(0, _p)

import numpy as np
import ml_dtypes

import concourse.bass as bass
import concourse.mybir as mybir
import concourse.tile as tile
from concourse import bacc
from concourse.bass import ds, ts
from concourse.bass_utils import run_bass_kernel_spmd
from concourse.masks import make_identity

BF16 = mybir.dt.bfloat16
F32 = mybir.dt.float32
I32 = mybir.dt.int32

V, H, T_TAGS, B, S_FULL = 100000, 256, 11, 32, 512
START, STOP = 9, 10
NCORES = 8
BL = B // NCORES  # 4 batch rows per core
AF = mybir.ActivationFunctionType
ALU = mybir.AluOpType
TT = T_TAGS
PF = 8  # xg prefetch depth (steps per DMA)


def _bcast_free(ap, n, pos=1):
    """Insert a broadcast (step 0, count n) free dim at position pos."""
    newap = ap.ap[:pos] + [[0, n]] + ap.ap[pos:]
    return bass.AP(ap.tensor, ap.offset, newap)


def build_nc(S):
    NCH = 4
    SC = S // NCH  # time-chunk size
    TPB = S // 128  # gather chunks per batch row
    NIDX = BL * TPB
    nc = bacc.Bacc()

    # ---------------- DRAM parameters ----------------
    d_idx = nc.dram_tensor("idx", [128, NIDX], I32, kind="ExternalInput")
    d_emb = nc.dram_tensor("emb", [V, H], F32, kind="ExternalInput")
    dw = {}
    for L, K in ((0, H), (1, 2 * H)):
        for d in "fb":
            dw[f"wih{L}{d}"] = nc.dram_tensor(f"wih{L}{d}", [K, 4 * H], F32,
                                              kind="ExternalInput")
            dw[f"whh{L}{d}"] = nc.dram_tensor(f"whh{L}{d}", [H, 4 * H], F32,
                                              kind="ExternalInput")
            dw[f"b{L}{d}"] = nc.dram_tensor(f"b{L}{d}", [1, 8 * 128], F32,
                                            kind="ExternalInput")
    d_woutT = nc.dram_tensor("woutT", [2 * H, TT], F32, kind="ExternalInput")
    d_bout = nc.dram_tensor("bout", [1, TT], F32, kind="ExternalInput")
    d_transrep = nc.dram_tensor("transrep", [1, TT * TT], F32,
                                kind="ExternalInput")
    d_transTrep = nc.dram_tensor("transTrep", [1, TT * TT], F32,
                                 kind="ExternalInput")
    d_transstart = nc.dram_tensor("transstart", [1, TT], F32,
                                  kind="ExternalInput")
    d_transstop = nc.dram_tensor("transstop", [1, TT], F32,
                                 kind="ExternalInput")
    d_iotam = nc.dram_tensor("iotam", [1, TT], F32, kind="ExternalInput")
    d_score = nc.dram_tensor("score_out", [BL, 1], F32, kind="ExternalOutput")
    d_tag = nc.dram_tensor("tag_out", [BL, S], I32, kind="ExternalOutput")

    # internal DRAM for gate inputs: [128, 8, S, BL] f32, (t,b) contig per m
    d_xg = {}
    for L in (0, 1):
        for d in "fb":
            d_xg[f"{L}{d}"] = nc.dram_tensor(f"xg{L}{d}", [128, 8, S, BL],
                                             F32)

    with tile.TileContext(nc) as tc:
        with (
            tc.tile_pool(name="const", bufs=1) as constp,
            tc.tile_pool(name="tpsum", bufs=2,
                         space=bass.MemorySpace.PSUM) as tpsump,
            tc.tile_pool(name="scanps", bufs=1,
                         space=bass.MemorySpace.PSUM) as scanpsp,
            tc.tile_pool(name="projps", bufs=2,
                         space=bass.MemorySpace.PSUM) as projpsp,
            tc.tile_pool(name="step", bufs=3) as stepp,
            tc.tile_pool(name="stage", bufs=2) as stagep,
            tc.tile_pool(name="pfpool", bufs=3) as pfp,
            tc.tile_pool(name="vit", bufs=4) as vitp,
            tc.tile_pool(name="wts", bufs=1) as wtsp,
            tc.tile_pool(name="hseq", bufs=1) as hseqp,
            tc.tile_pool(name="bigp", bufs=1) as bigp,
        ):
            # ---------------- P0: constants ----------------
            ident = constp.tile([128, 128], F32, tag="ident")
            make_identity(nc, ident[:])

            sb_idx = constp.tile([128, NIDX], I32, tag="sbidx")
            nc.sync.dma_start(sb_idx[:], d_idx[:])

            woutT = constp.tile([128, 4, TT], F32, tag="woutT")
            for k in range(4):
                nc.sync.dma_start(woutT[:, k, :], d_woutT[ts(k, 128), :])
            bout = constp.tile([1, TT], F32, tag="bout")
            nc.sync.dma_start(bout[:], d_bout[:])
            onesrow = constp.tile([1, 512], F32, tag="onesrow")
            nc.vector.memset(onesrow[:], 1.0)

            biases = {}
            for L in (0, 1):
                for d in "fb":
                    bt = constp.tile([1, 8, 128], F32, tag=f"b{L}{d}",
                                     name=f"b{L}{d}")
                    nc.sync.dma_start(bt[:].rearrange("p m q -> p (m q)"),
                                      dw[f"b{L}{d}"][:])
                    biases[f"{L}{d}"] = bt

            def brd(dram, n, tg):
                t = constp.tile([BL, n], F32, tag=tg, name=tg)
                src = bass.AP(dram, 0, [[0, BL], [1, n]])
                nc.gpsimd.dma_start(t[:], src)
                return t

            transrep = brd(d_transrep, TT * TT, "trep")
            transTrep = brd(d_transTrep, TT * TT, "tTrep")
            transstart = brd(d_transstart, TT, "tstart")
            transstop = brd(d_transstop, TT, "tstop")
            iotam = brd(d_iotam, TT, "iotam")

            # ACT table preload (sigmoid_and_others holds sigmoid+tanh)
            warm = constp.tile([1, 8], F32, tag="warm")
            nc.vector.memset(warm[:], 0.0)
            nc.scalar.activation(warm[:], warm[:], AF.Sigmoid)

            # chunk processing order: serve fwd chunk0 / bwd chunk last first
            chorder = [0, NCH - 1] + list(range(1, NCH - 1))

            # LSTM weight slots, shared across layers via tags
            def load_w(kind, L, d):
                dram = dw[f"{kind}{L}{d}"]
                nk = dram.shape[0] // 128
                wt = wtsp.tile([128, nk, 4 * H], F32, tag=f"{kind}_{d}",
                               name=f"{kind}{L}{d}")
                for k in range(nk):
                    nc.sync.dma_start(wt[:, k, :], dram[ts(k, 128), :])
                return wt

            # ---------------- P1: embedding gather + transpose ----------
            xeT = bigp.tile([128, 2, S, BL], F32, tag="bigA",
                            name="xeT")
            jorder = [j for nchunk in chorder for j in range(NIDX)
                      if ((j % TPB) * 128) // SC == nchunk]
            for j in jorder:
                bb, t0 = j // TPB, 128 * (j % TPB)
                xch = stagep.tile([128, H], F32, tag="xch", name="xch")
                nc.gpsimd.indirect_dma_start(
                    out=xch[:], out_offset=None, in_=d_emb[:],
                    in_offset=bass.IndirectOffsetOnAxis(ap=sb_idx[:, j:j + 1],
                                                        axis=0),
                )
                for hh in range(2):
                    tp = tpsump.tile([128, 128], F32, tag="tp", name="tp")
                    nc.tensor.transpose(tp[:], xch[:, ts(hh, 128)], ident[:])
                    nc.vector.tensor_copy(xeT[:, hh, t0:t0 + 128, bb], tp[:])

            # ---------------- bulk gate-input projection -> DRAM --------
            def project(wihT, bias, srcs, dxg):
                for nchunk in chorder:
                    for m in range(8):
                        ps = projpsp.tile([128, SC * BL], F32, tag="projps",
                                          name="ps")
                        nc.tensor.matmul(
                            ps[:], bias[:, m, :], onesrow[:, :SC * BL],
                            start=True, stop=False,
                        )
                        for ki, (st, k) in enumerate(srcs):
                            stt = st[nchunk] if isinstance(st, list) else st
                            mv = (stt[:, k, :, :] if isinstance(st, list)
                                  else stt[:, k, nchunk * SC:(nchunk + 1) * SC,
                                           :])
                            nc.tensor.matmul(
                                ps[:], wihT[:, ki, ts(m, 128)], mv,
                                start=False, stop=(ki == len(srcs) - 1),
                            )
                        stg = stagep.tile([128, SC * BL], F32, tag="stg",
                                          name="stg")
                        nc.vector.tensor_copy(stg[:], ps[:])
                        nc.sync.dma_start(
                            dxg[:, m, nchunk * SC:(nchunk + 1) * SC, :],
                            stg[:])

            # ---------------- one LSTM layer: fwd+bwd scans --------------
            def scan_layer(L, hf, hb, whhf, whhb):
                dxgf, dxgb = d_xg[f"{L}f"], d_xg[f"{L}b"]
                cst = {"f": None, "b": None}
                pft = {"f": None, "b": None}
                for u in range(S):
                    for d, hseq, whh, dxg in (("f", hf, whhf, dxgf),
                                              ("b", hb, whhb, dxgb)):
                        t = u if d == "f" else S - 1 - u
                        ch, tt_ = t // SC, t % SC
                        if u % PF == 0:
                            blk = t // PF
                            pf = pfp.tile([128, 8, PF, BL], F32,
                                          tag=f"pf{d}", name=f"pf{d}")
                            nc.sync.dma_start(
                                pf[:],
                                dxg[:, :, blk * PF:(blk + 1) * PF, :])
                            pft[d] = pf
                        ps = scanpsp.tile([128, 8, BL], F32, tag=f"g{d}",
                                          name=f"g{d}")
                        nc.tensor.matmul(
                            ps[:], ident[:], pft[d][:, :, t % PF, :],
                            start=True, stop=(u == 0),
                            skip_group_check=True,
                        )
                        if u > 0:
                            tp_ = t - 1 if d == "f" else t + 1
                            pch, ptt = tp_ // SC, tp_ % SC
                            for m in range(8):
                                for k in range(2):
                                    nc.tensor.matmul(
                                        ps[:, m, :],
                                        whh[:, k, ts(m, 128)],
                                        hseq[pch][:, k, ptt, :],
                                        start=False, stop=(k == 1),
                                        skip_group_check=True,
                                    )
                        sig = stepp.tile([128, 8, BL], F32, tag=f"sig{d}",
                                         name=f"sig{d}")
                        nc.scalar.activation(sig[:, 0:6, :], ps[:, 0:6, :],
                                             AF.Sigmoid)
                        nc.scalar.activation(sig[:, 6:8, :], ps[:, 6:8, :],
                                             AF.Tanh)
                        m1 = stepp.tile([128, 2, BL], F32, tag=f"m1{d}",
                                        name=f"m1{d}")
                        nc.vector.tensor_tensor(m1[:], sig[:, 0:2, :],
                                                sig[:, 6:8, :], ALU.mult)
                        cn = stepp.tile([128, 2, BL], F32, tag=f"c{d}",
                                        name=f"c{d}")
                        if cst[d] is None:
                            nc.vector.tensor_copy(cn[:], m1[:])
                        else:
                            m2 = stepp.tile([128, 2, BL], F32, tag=f"m2{d}",
                                            name=f"m2{d}")
                            nc.vector.tensor_tensor(m2[:], sig[:, 2:4, :],
                                                    cst[d][:], ALU.mult)
                            nc.vector.tensor_tensor(cn[:], m1[:], m2[:],
                                                    ALU.add)
                        cst[d] = cn
                        th = stepp.tile([128, 2, BL], F32, tag=f"th{d}",
                                        name=f"th{d}")
                        nc.scalar.activation(th[:], cn[:], AF.Tanh)
                        nc.vector.tensor_tensor(hseq[ch][:, :, tt_, :],
                                                sig[:, 4:6, :], th[:],
                                                ALU.mult)

            # ---------------- P2+P3: layer 0 ----------------
            wih0f = load_w("wih", 0, "f")
            wih0b = load_w("wih", 0, "b")
            project(wih0f, biases["0f"], [(xeT, 0), (xeT, 1)], d_xg["0f"])
            project(wih0b, biases["0b"], [(xeT, 0), (xeT, 1)], d_xg["0b"])

            whh0f = load_w("whh", 0, "f")
            whh0b = load_w("whh", 0, "b")
            h0f = [hseqp.tile([128, 2, SC, BL], F32, tag=f"h_f{i}",
                              name=f"h0f{i}") for i in range(NCH)]
            h0b = [hseqp.tile([128, 2, SC, BL], F32, tag=f"h_b{i}",
                              name=f"h0b{i}") for i in range(NCH)]
            scan_layer(0, h0f, h0b, whh0f, whh0b)

            # ---------------- P4+P5: layer 1 ----------------
            wih1f = load_w("wih", 1, "f")
            wih1b = load_w("wih", 1, "b")
            srcs1 = [(h0f, 0), (h0f, 1), (h0b, 0), (h0b, 1)]
            project(wih1f, biases["1f"], srcs1, d_xg["1f"])
            project(wih1b, biases["1b"], srcs1, d_xg["1b"])

            whh1f = load_w("whh", 1, "f")
            whh1b = load_w("whh", 1, "b")
            h1f = [hseqp.tile([128, 2, SC, BL], F32, tag=f"h_f{i}",
                              name=f"h1f{i}") for i in range(NCH)]
            h1b = [hseqp.tile([128, 2, SC, BL], F32, tag=f"h_b{i}",
                              name=f"h1b{i}") for i in range(NCH)]
            scan_layer(1, h1f, h1b, whh1f, whh1b)

            # ---------------- P6: feats ----------------
            featsT = bigp.tile([TT, S * BL], F32, tag="featsT",
                               name="featsT")
            for nchunk in range(NCH):
                ps = projpsp.tile([TT, SC * BL], F32, tag="projps", name="ps")
                srcs = [(h1f, 0), (h1f, 1), (h1b, 0), (h1b, 1)]
                nc.tensor.matmul(ps[:], bout[:], onesrow[:, :SC * BL],
                                 start=True, stop=False)
                for ki, (st, k) in enumerate(srcs):
                    nc.tensor.matmul(
                        ps[:], woutT[:, ki, :], st[nchunk][:, k, :, :],
                        start=False, stop=(ki == 3),
                    )
                nc.vector.tensor_copy(
                    featsT[:, nchunk * SC * BL:(nchunk + 1) * SC * BL],
                    ps[:])

            # transpose to [(t,b), j] blocks then DMA to [b, (t,j)]
            NBLK = (S * BL) // 128
            featsTT = bigp.tile([128, NBLK, TT], F32, tag="featsTT",
                                name="featsTT")
            for blk in range(NBLK):
                tp = tpsump.tile([128, TT], F32, tag="tp", name="tp")
                nc.tensor.transpose(tp[:], featsT[:, ts(blk, 128)],
                                    ident[:TT, :TT])
                nc.vector.tensor_copy(featsTT[:, blk, :], tp[:])
            feats_v = bigp.tile([BL, S * TT], F32, tag="bigB",
                                name="featsv")
            TPB2 = 128 // BL  # 32 t' rows per block
            for bb in range(BL):
                for blk in range(NBLK):
                    src = featsTT[bb::BL, blk, :]  # [TPB2, TT]
                    dbase = feats_v[bb:bb + 1,
                                    blk * TPB2 * TT:(blk + 1) * TPB2 * TT]
                    dst = bass.AP(dbase.tensor, dbase.offset,
                                  [dbase.ap[0], [TT, TPB2], [1, TT]])
                    nc.sync.dma_start(dst, src)

            # ---------------- P7: Viterbi scans ----------------
            fv = bigp.tile([BL, S * TT], F32, tag="bigA", name="fv")
            bv = bigp.tile([BL, S * TT], F32, tag="bigC", name="bv")
            for u in range(S):
                t = u  # forward
                if t == 0:
                    nc.vector.tensor_tensor(fv[:, 0:TT], feats_v[:, 0:TT],
                                            transstart[:], ALU.add)
                else:
                    sc = vitp.tile([BL, TT, TT], F32, tag="scf", name="scf")
                    prev = _bcast_free(fv[:, (t - 1) * TT:t * TT], TT, pos=1)
                    nc.vector.tensor_tensor(
                        sc[:], prev,
                        transrep[:].rearrange("p (j i) -> p j i", i=TT),
                        ALU.add)
                    mx = vitp.tile([BL, TT], F32, tag="mxf", name="mxf")
                    nc.vector.tensor_reduce(mx[:], sc[:],
                                            mybir.AxisListType.X, ALU.max)
                    nc.vector.tensor_tensor(fv[:, t * TT:(t + 1) * TT], mx[:],
                                            feats_v[:, t * TT:(t + 1) * TT],
                                            ALU.add)
                t = S - 1 - u  # backward
                if t == S - 1:
                    nc.vector.tensor_tensor(bv[:, t * TT:(t + 1) * TT],
                                            feats_v[:, t * TT:(t + 1) * TT],
                                            transstop[:], ALU.add)
                else:
                    sc = vitp.tile([BL, TT, TT], F32, tag="scb", name="scb")
                    nxt = _bcast_free(bv[:, (t + 1) * TT:(t + 2) * TT], TT,
                                      pos=1)
                    nc.vector.tensor_tensor(
                        sc[:], nxt,
                        transTrep[:].rearrange("p (i j) -> p i j", j=TT),
                        ALU.add)
                    mx = vitp.tile([BL, TT], F32, tag="mxb", name="mxb")
                    nc.vector.tensor_reduce(mx[:], sc[:],
                                            mybir.AxisListType.X, ALU.max)
                    nc.vector.tensor_tensor(bv[:, t * TT:(t + 1) * TT], mx[:],
                                            feats_v[:, t * TT:(t + 1) * TT],
                                            ALU.add)

            # ---------------- P8: score + tags ----------------
            term = bigp.tile([BL, TT], F32, tag="term", name="term")
            nc.vector.tensor_tensor(term[:], fv[:, (S - 1) * TT:S * TT],
                                    transstop[:], ALU.add)
            score_sb = bigp.tile([BL, 1], F32, tag="score", name="score")
            nc.vector.tensor_reduce(score_sb[:], term[:],
                                    mybir.AxisListType.X, ALU.max)
            nc.sync.dma_start(d_score[:], score_sb[:])

            # mu = fv + bv - feats (reuse fv); eq -> bv; sel -> feats_v
            nc.vector.tensor_tensor(fv[:], fv[:], bv[:], ALU.add)
            nc.vector.tensor_tensor(fv[:], fv[:], feats_v[:], ALU.subtract)
            mxt = bigp.tile([BL, S], F32, tag="mxt", name="mxt")
            nc.vector.tensor_reduce(
                mxt[:], fv[:].rearrange("p (t j) -> p t j", j=TT),
                mybir.AxisListType.X, ALU.max)
            mxb = _bcast_free(mxt[:], TT, pos=2)
            nc.vector.tensor_tensor(bv[:].rearrange("p (t j) -> p t j", j=TT),
                                    fv[:].rearrange("p (t j) -> p t j", j=TT),
                                    mxb, ALU.is_equal)
            iob = _bcast_free(iotam[:], S, pos=1)
            nc.vector.tensor_tensor(
                feats_v[:].rearrange("p (t j) -> p t j", j=TT),
                bv[:].rearrange("p (t j) -> p t j", j=TT), iob, ALU.mult)
            tagf = bigp.tile([BL, S], F32, tag="tagf", name="tagf")
            nc.vector.tensor_reduce(
                tagf[:], feats_v[:].rearrange("p (t j) -> p t j", j=TT),
                mybir.AxisListType.X, ALU.min)
            tagi = bigp.tile([BL, S], I32, tag="tagi", name="tagi")
            nc.vector.tensor_scalar(tagi[:], tagf[:], 1000.0, None, ALU.add)
            nc.sync.dma_start(d_tag[:], tagi[:])

    nc.compile()
    return nc


# gate permutation: pytorch (i,f,g,o) -> (i,f,o,g)
_PERM = np.r_[0:256, 256:512, 768:1024, 512:768]


def _prep_weights(inputs, S):
    f32 = np.float32
    out = {}
    out["emb"] = np.ascontiguousarray(np.asarray(inputs["emb"], f32))
    for L in (0, 1):
        for d in "fb":
            sfx = f"{L}{d}"
            wih = np.asarray(inputs[f"wih{sfx}"], f32)[_PERM]
            whh = np.asarray(inputs[f"whh{sfx}"], f32)[_PERM]
            bb = np.asarray(inputs[f"b{sfx}"], f32)[_PERM]
            out[f"wih{sfx}"] = np.ascontiguousarray(wih.T)
            out[f"whh{sfx}"] = np.ascontiguousarray(whh.T)
            out[f"b{sfx}"] = np.ascontiguousarray(bb.reshape(1, 8 * 128))
    out["woutT"] = np.ascontiguousarray(np.asarray(inputs["w_out"], f32).T)
    out["bout"] = np.asarray(inputs["b_out"], f32).reshape(1, TT).copy()
    trans = np.asarray(inputs["trans"], f32)
    out["transrep"] = trans.reshape(1, -1).copy()
    out["transTrep"] = np.ascontiguousarray(trans.T).reshape(1, -1).copy()
    out["transstart"] = np.ascontiguousarray(trans[:, START]).reshape(1, -1)
    out["transstop"] = np.ascontiguousarray(trans[STOP, :]).reshape(1, -1)
    out["iotam"] = (np.arange(TT, dtype=f32) - 1000.0).reshape(1, -1)
    return out


def _prep_core(x_core, S):
    """Per-core gather-index layout: idx[p, j] = x[j//TPB, 128*(j%TPB)+p]."""
    TPB = S // 128
    nidx = BL * TPB
    idx = np.empty((128, nidx), np.int32)
    for j in range(nidx):
        idx[:, j] = x_core[j // TPB, 128 * (j % TPB):128 * (j % TPB) + 128]
    return idx


_NC_CACHE = {}
LAST_EXEC_NS = None


def _ensure_ntff_hook():
    try:
        from antenv import axon_hooks  # noqa: F401
        return True
    except ImportError:
        pass
    try:
        import types
        import antenv
        if "/root/.axon_site" not in sys.path:
            sys.path.insert(0, "/root/.axon_site")
        from trn_agent_boot.trn_boot import _ntff_profile_via_ctypes
        hook = _ntff_profile_via_ctypes("/opt/axon/libaxon_pjrt.so")
        if hook is None:
            return False
        mod = types.ModuleType("antenv.axon_hooks")
        state = {"hook": hook}
        mod.set_axon_ntff_profile_hook = lambda h: state.__setitem__("hook", h)
        mod.get_axon_ntff_profile_hook = lambda: state["hook"]
        sys.modules["antenv.axon_hooks"] = mod
        antenv.axon_hooks = mod
        return True
    except Exception as e:  # noqa: BLE001
        print("ntff hook shim failed:", e)
        return False


def _run(inputs, S):
    global LAST_EXEC_NS
    if S not in _NC_CACHE:
        _NC_CACHE[S] = build_nc(S)
    nc = _NC_CACHE[S]
    shared = _prep_weights(inputs, S)
    x = np.asarray(np.asarray(inputs["x"], np.int64), np.int32)
    in_maps = []
    for c in range(NCORES):
        m = dict(shared)
        m["idx"] = _prep_core(x[c * BL:(c + 1) * BL], S)
        in_maps.append(m)
    trace = bool(os.environ.get("KERNEL_TRACE")) and _ensure_ntff_hook()
    try:
        res = run_bass_kernel_spmd(nc, in_maps, list(range(NCORES)),
                                   trace=trace)
    except Exception:
        if not trace:
            raise
        res = run_bass_kernel_spmd(nc, in_maps, list(range(NCORES)),
                                   trace=False)
    LAST_EXEC_NS = res.exec_time_ns
    scores = np.concatenate(
        [np.asarray(r["score_out"], np.float32).reshape(BL)
         for r in res.results])
    tags = np.concatenate(
        [np.asarray(r["tag_out"]).reshape(BL, S) for r in res.results])
    return scores.astype(np.float32), tags.astype(np.int32)


def kernel(x, batch_seq_len, emb, wih0f, whh0f, b0f, wih0b, whh0b, b0b,
           wih1f, whh1f, b1f, wih1b, whh1b, b1b, w_out, b_out, trans):
    inputs = dict(x=x, emb=emb, wih0f=wih0f, whh0f=whh0f, b0f=b0f,
                  wih0b=wih0b, whh0b=whh0b, b0b=b0b, wih1f=wih1f,
                  whh1f=whh1f, b1f=b1f, wih1b=wih1b, whh1b=whh1b, b1b=b1b,
                  w_out=w_out, b_out=b_out, trans=trans)
    S = int(np.asarray(x).shape[1])
    return _run(inputs, S)
